# revision 1
# baseline (speedup 1.0000x reference)
"""Trainium2 Bass kernel for MultiHeadedSelfAttention (fastexp softmax).

Sharding: sequence-parallel over 8 cores. Each core computes K/V for the
full sequence and attention for its own 512-row query block; outputs are
disjoint row blocks of the final [4096, 512] result, so no collectives.

Device layout is "transposed everything": activations stored feature-major
(d on partitions) so projections and scores feed the PE contraction dim
directly. Softmax row-max is computed from an [l, m]-oriented score pass
(DVE reduce), then scores are recomputed transposed [m, l] with the row max
injected as an extra contraction row (ones x -mu), so the exponent input
arrives in PSUM already max-subtracted. The Schraudolph fastexp is applied
bit-exactly: ACT affine (scale=A, bias=B) with int32 output = the reference's
int(A*x+B); the int32 bit pattern viewed as f32 is e. GPSIMD converts e to
fp16 for the PV matmul. Row sums come free as a ones column appended to V.
"""

import numpy as np

DIM = 512
H = 8
HD = 64
SEQ = 4096
NCORES = 8
LSP = SEQ // NCORES  # 512 query rows per core

GIST_A = 12102203.17133801
GIST_B = 1064986823.0


def _q_rsqrt(x):
    y = np.asarray((x,), dtype=np.float32)
    x2 = y * 0.5
    i = y.view(np.int32)
    i = np.right_shift(i, 1)
    i = 1597463007 - i
    y = i.view(np.float32)
    y = y * (1.5 - x2 * y * y)
    return float(y[0])


SCALING = _q_rsqrt(HD)

_CACHE = {}


EMODE = "f16t"         # "f16": GP-converted fp16 e; "bf16": int16-trick bf16 e
EXP_DVE_EVERY = 0      # every Nth exp tile on DVE instead of ACT (0 = all ACT)
EVAC_DVE = False       # v evacuations on DVE instead of ACT
KEVAC_DVE = False      # kT evacuations on DVE (tensor_scalar add)
XT_SPLIT = False       # split xT input DMA into 4 chunk DMAs
PSO2 = False           # merge psM into psO with bufs=2
EXP_PAIR = False       # exp per [128,1024] psum span (psB coarse, psA bufs=1)
ATEVAC_DVE = True      # aT evacuation on DVE
OTEVAC_DVE = False     # oT evacuation on DVE
MUEVAC_DVE = True      # mu-row evacuation on DVE
SHIFT_GPDMA = False    # kT/qT partition-shift DMAs via SWDGE (gpsimd)
DMA_REORDER = True     # wk first, xT split into chunks
VPROJ_AB = False       # v-proj alternates psA/psB slots, before stats
DMA_FINE = False       # wk per-dc + xT half-chunks, alternating DMA engines
ST_ALT3 = False        # every 3rd sT tile allocates from psM (3-deep sT)
XT_JMAJOR = False      # xT DMA in 8 j-span slices (consumption order)
KSHIFT_PAIR = True     # kT evac+shift in j-pairs (32 shift DMAs, tmp 1024)
PROJ_INTERLEAVE = False  # p-outer kT loop; q+stats emitted per pair
EXP_DVE_FROM_H = 99    # route exp to DVE for heads >= this (DVE idle tail)
Y_EARLY = False        # emit y pair-partials 0-2 as fillers in the h=7 round
V_AFTER_STATS = True   # emit v projection after stats(0)/stats(1)
STARTUP_IL = True      # dc-interleave the first 8 kT tiles across idle psum
KSHIFT_QUAD = False    # 4-span kT staging (fewer shift DMAs, tmp bufs 3)


def _build():
    key = ("nc", EMODE, EXP_DVE_EVERY, EVAC_DVE, V_AFTER_STATS, KEVAC_DVE, XT_SPLIT, PSO2, EXP_PAIR, ATEVAC_DVE, OTEVAC_DVE, MUEVAC_DVE, SHIFT_GPDMA, DMA_REORDER, VPROJ_AB, DMA_FINE, ST_ALT3, XT_JMAJOR, KSHIFT_PAIR, PROJ_INTERLEAVE, EXP_DVE_FROM_H, Y_EARLY, STARTUP_IL, KSHIFT_QUAD)
    if key in _CACHE:
        return _CACHE[key]

    import concourse.bass as bass
    import concourse.mybir as mybir
    import concourse.tile as tile
    from concourse import bacc
    from concourse.masks import make_identity

    f16 = mybir.dt.float16
    bf16 = mybir.dt.bfloat16
    f32 = mybir.dt.float32
    i32 = mybir.dt.int32
    i16 = mybir.dt.int16
    AF = mybir.ActivationFunctionType
    ALU = mybir.AluOpType

    f16mode = EMODE == "f16"
    if f16mode:
        vdt, edt, expA, expB = f16, i32, GIST_A, GIST_B
    elif EMODE == "bf16":
        vdt, edt = bf16, i16
        expA, expB = GIST_A / 65536.0, GIST_B / 65536.0
    else:  # "f16t": Schraudolph applied directly in the fp16 bit domain
        vdt, edt = f16, i16
        expA = 1024.0 / float(np.log(2.0))
        expB = 15.0 * 1024.0 + (GIST_B / 8192.0 - 130048.0)

    nc = bacc.Bacc("TRN2", target_bir_lowering=False, debug=False,
                   num_devices=NCORES)

    d_xT = nc.dram_tensor("xT", (128, 4, SEQ), f16, kind="ExternalInput")
    d_xqT = nc.dram_tensor("xqT", (128, 4, LSP), f16, kind="ExternalInput")
    d_wq = nc.dram_tensor("wq", (128, 4, DIM), f16, kind="ExternalInput")
    d_wk = nc.dram_tensor("wk", (128, 4, DIM), f16, kind="ExternalInput")
    d_wv = nc.dram_tensor("wv", (128, 4, DIM), f16, kind="ExternalInput")
    d_wo = nc.dram_tensor("wo", (128, 4, DIM), f16, kind="ExternalInput")
    d_bqp = nc.dram_tensor("bqp", (128, 4), f32, kind="ExternalInput")
    d_bkp = nc.dram_tensor("bkp", (128, 4), f32, kind="ExternalInput")
    d_crow = nc.dram_tensor("crow", (DIM,), f32, kind="ExternalInput")
    d_ones = nc.dram_tensor("ones16", (H, SEQ), f16, kind="ExternalInput")
    d_y = nc.dram_tensor("y", (LSP, DIM), f32, kind="ExternalOutput")

    MSP = SEQ // 128  # 32 m chunks
    exp_ctr = [0]

    with tile.TileContext(nc) as tc:
        with (
            tc.tile_pool(name="const", bufs=1) as cp,
            tc.tile_pool(name="big", bufs=1) as bp,
            tc.tile_pool(name="tmp", bufs=3 if KSHIFT_QUAD else 6) as tp,
            tc.tile_pool(name="small", bufs=4) as sp,
            tc.tile_pool(name="t32p", bufs=4) as t32p,
            tc.tile_pool(name="e16p", bufs=3) as e16p,
            tc.tile_pool(name="psA", bufs=1 if EXP_PAIR else 2,
                         space="PSUM") as psA,
            tc.tile_pool(name="psB", bufs=2, space="PSUM") as psB,
            tc.tile_pool(name="psO", bufs=2 if PSO2 else 1,
                         space="PSUM") as psO,
            tc.tile_pool(name="psM", bufs=1, space="PSUM") as psM_pool,
        ):
            # ---- constants / inputs in SBUF
            kT_aug = bp.tile([65, H, SEQ], f16)     # per-head k^T + ones row
            xT = cp.tile([128, 4, SEQ], f16)
            xqT = cp.tile([128, 4, LSP], f16)
            wq = cp.tile([128, 4, DIM], f16)
            wk = cp.tile([128, 4, DIM], f16)
            wv = cp.tile([128, 4, DIM], f16)
            wo = cp.tile([128, 4, DIM], f16)
            bqp = cp.tile([128, 4], f32)
            bkp = cp.tile([128, 4], f32)
            crow_b = cp.tile([128, DIM], f32)
            biasB = cp.tile([128, 1], f32)
            idf32 = cp.tile([128, 128], f32)
            idf16 = cp.tile([128, 128], f16)
            negI = cp.tile([128, 128], f16)

            if DMA_FINE:
                for dc in range(4):
                    nc.sync.dma_start(out=wk[:, dc, :], in_=d_wk[:, dc, :])
                for dc in range(4):
                    for hh in range(2):
                        eng = nc.sync if (dc + hh) % 2 == 0 else nc.gpsimd
                        sl = slice(2048 * hh, 2048 * hh + 2048)
                        eng.dma_start(out=xT[:, dc, sl], in_=d_xT[:, dc, sl])
                nc.sync.dma_start(out=wv, in_=d_wv[:, :, :])
                nc.sync.dma_start(out=wq, in_=d_wq[:, :, :])
                nc.sync.dma_start(out=xqT, in_=d_xqT[:, :, :])
                nc.sync.dma_start(out=wo, in_=d_wo[:, :, :])
            elif DMA_REORDER:
                nc.sync.dma_start(out=wk, in_=d_wk[:, :, :])
                nc.sync.dma_start(out=bkp, in_=d_bkp[:, :])
                if STARTUP_IL:
                    nc.sync.dma_start(out=wq, in_=d_wq[:, :, :])
                    nc.sync.dma_start(out=bqp, in_=d_bqp[:, :])
                    nc.sync.dma_start(out=xqT, in_=d_xqT[:, :, :])
                    nc.sync.dma_start(out=kT_aug[64:65, :, :], in_=d_ones[:, :])
                if XT_JMAJOR:
                    for j in range(8):
                        sl = slice(512 * j, 512 * j + 512)
                        nc.sync.dma_start(out=xT[:, :, sl], in_=d_xT[:, :, sl])
                else:
                    for dc in range(4):
                        nc.sync.dma_start(out=xT[:, dc, :], in_=d_xT[:, dc, :])
                nc.sync.dma_start(out=wv, in_=d_wv[:, :, :])
                if not STARTUP_IL:
                    nc.sync.dma_start(out=wq, in_=d_wq[:, :, :])
                    nc.sync.dma_start(out=xqT, in_=d_xqT[:, :, :])
                nc.sync.dma_start(out=wo, in_=d_wo[:, :, :])
            else:
                if XT_SPLIT:
                    for dc in range(4):
                        nc.sync.dma_start(out=xT[:, dc, :], in_=d_xT[:, dc, :])
                else:
                    nc.sync.dma_start(out=xT, in_=d_xT[:, :, :])
                nc.sync.dma_start(out=xqT, in_=d_xqT[:, :, :])
                nc.sync.dma_start(out=wq, in_=d_wq[:, :, :])
                nc.sync.dma_start(out=wk, in_=d_wk[:, :, :])
                nc.sync.dma_start(out=wv, in_=d_wv[:, :, :])
                nc.sync.dma_start(out=wo, in_=d_wo[:, :, :])
            if not DMA_REORDER:
                nc.sync.dma_start(out=bqp, in_=d_bqp[:, :])
                nc.sync.dma_start(out=bkp, in_=d_bkp[:, :])
            elif not STARTUP_IL:
                nc.sync.dma_start(out=bqp, in_=d_bqp[:, :])
            crow_ap = d_crow[:]
            crow_bcast = bass.AP(tensor=crow_ap.tensor, offset=crow_ap.offset,
                                 ap=[[0, 128]] + list(crow_ap.ap))
            nc.sync.dma_start(out=crow_b, in_=crow_bcast)

            nc.vector.memset(biasB, expB)
            make_identity(nc, idf32)
            make_identity(nc, idf16)
            nc.scalar.mul(negI, idf16, -1.0)

            # ---- persistent activations (kT_aug declared above the DMAs)
            qT_aug = bp.tile([65, H, LSP], f16)     # per-head q^T + (-mu) row
            v_sb = bp.tile([128, MSP, H, 65], vdt)  # v + ones col, m-major
            attnT = bp.tile([128, 4, LSP], f16)
            if not (DMA_REORDER and STARTUP_IL):
                nc.sync.dma_start(out=kT_aug[64:65, :, :], in_=d_ones[:, :])
            nc.vector.memset(v_sb[:, :, :, 64:65], 1.0)

            # ---- projections
            shift_eng = nc.gpsimd if SHIFT_GPDMA else nc.sync

            def emit_kpair(p, jp):
                # jp now indexes a 4-span group when KSHIFT_QUAD
                width = 4 if KSHIFT_QUAD else 2
                tmp = tp.tile([128, 512 * width], f16, tag="tmp", name="tmpk")
                for k in range(width):
                    j = width * jp + k
                    kp = psB.tile([128, 512], f32, tag="psB", name="kp")
                    for dc in range(4):
                        nc.tensor.matmul(
                            kp, wk[:, dc, 128 * p:128 * p + 128],
                            xT[:, dc, 512 * j:512 * j + 512],
                            start=(dc == 0), stop=(dc == 3))
                    nc.scalar.activation(
                        out=tmp[:, 512 * k:512 * k + 512], in_=kp,
                        func=AF.Identity, bias=bkp[:, p:p + 1],
                        scale=1.0)
                w512 = 512 * (4 if KSHIFT_QUAD else 2)
                sl = slice(w512 * jp, w512 * jp + w512)
                shift_eng.dma_start(out=kT_aug[0:64, 2 * p, sl],
                                    in_=tmp[0:64, :])
                shift_eng.dma_start(out=kT_aug[0:64, 2 * p + 1, sl],
                                    in_=tmp[64:128, :])

            def emit_kquad_rest(p):
                for j0, width in ((2, 4), (6, 2)):
                    tmp = tp.tile([128, 512 * width], f16, tag="tmp",
                                  name="tmpk")
                    for k in range(width):
                        j = j0 + k
                        kp = psB.tile([128, 512], f32, tag="psB", name="kp")
                        for dc in range(4):
                            nc.tensor.matmul(
                                kp, wk[:, dc, 128 * p:128 * p + 128],
                                xT[:, dc, 512 * j:512 * j + 512],
                                start=(dc == 0), stop=(dc == 3))
                        nc.scalar.activation(
                            out=tmp[:, 512 * k:512 * k + 512], in_=kp,
                            func=AF.Identity, bias=bkp[:, p:p + 1],
                            scale=1.0)
                    sl = slice(512 * j0, 512 * (j0 + width))
                    shift_eng.dma_start(out=kT_aug[0:64, 2 * p, sl],
                                        in_=tmp[0:64, :])
                    shift_eng.dma_start(out=kT_aug[0:64, 2 * p + 1, sl],
                                        in_=tmp[64:128, :])

            def emit_qpair(p):
                qp = psB.tile([128, 512], f32, tag="psB", name="qp")
                for dc in range(4):
                    nc.tensor.matmul(
                        qp, wq[:, dc, 128 * p:128 * p + 128],
                        xqT[:, dc, :], start=(dc == 0), stop=(dc == 3))
                tmp = tp.tile([128, 512], f16, tag="tmp")
                nc.scalar.activation(
                    out=tmp, in_=qp, func=AF.Identity,
                    bias=bqp[:, p:p + 1], scale=1.0)
                shift_eng.dma_start(out=qT_aug[0:64, 2 * p, :], in_=tmp[0:64, :])
                shift_eng.dma_start(out=qT_aug[0:64, 2 * p + 1, :],
                                    in_=tmp[64:128, :])
            if KSHIFT_PAIR and not PROJ_INTERLEAVE:
                if STARTUP_IL:
                    for p in range(4):
                        emit_qpair(p)
                    # jp=0 for all p: 8 tiles accumulate dc-interleaved so PE
                    # consumes each arriving xT chunk instead of waiting for
                    # the last one; borrows every pre-stats-idle psum bank.
                    f8 = []
                    psa_slots = [psA.tile([128, 1024], f32, tag="psA",
                                          name="ila"),
                                 psA.tile([128, 1024], f32, tag="psA",
                                          name="ilb")]
                    for p in range(4):
                        for k in range(2):
                            idx = 2 * p + k
                            if idx < 4:
                                t = psa_slots[idx // 2][:, 512 * (idx % 2):
                                                        512 * (idx % 2) + 512]
                            elif idx < 6:
                                t = psB.tile([128, 512], f32, tag="psB",
                                             name="ilc")
                            elif idx == 6:
                                t = psM_pool.tile([128, 512], f32, tag="psM",
                                                  name="ild")
                            else:
                                t = psO.tile([128, 512], f32, tag="psO",
                                             name="ile")
                            f8.append((t, p, k))
                    for dc in range(4):
                        for t, p, k in f8:
                            nc.tensor.matmul(
                                t, wk[:, dc, 128 * p:128 * p + 128],
                                xT[:, dc, 512 * k:512 * k + 512],
                                start=(dc == 0), stop=(dc == 3))
                    for p in range(4):
                        tmp = tp.tile([128, 1024], f16, tag="tmp", name="tmpk")
                        for k in range(2):
                            t = f8[2 * p + k][0]
                            nc.scalar.activation(
                                out=tmp[:, 512 * k:512 * k + 512], in_=t,
                                func=AF.Identity, bias=bkp[:, p:p + 1],
                                scale=1.0)
                        shift_eng.dma_start(out=kT_aug[0:64, 2 * p, 0:1024],
                                            in_=tmp[0:64, :])
                        shift_eng.dma_start(out=kT_aug[0:64, 2 * p + 1, 0:1024],
                                            in_=tmp[64:128, :])
                    if KSHIFT_QUAD:
                        # spans 2..7 remain: one quad group (2-5) + one pair-ish
                        # handled as group starting at jp such that width*jp=2
                        # simpler: two groups of width 3 not supported; use
                        # per-p: one [2..5] quad then [6..7] pair via direct
                        for p in range(4):
                            emit_kquad_rest(p)
                    else:
                        for p in range(4):
                            for jp in range(1, 4):
                                emit_kpair(p, jp)
                else:
                    for jp in range(4):   # j-pairs; j-outer for DMA arrival
                        for p in range(4):
                            emit_kpair(p, jp)
            elif not PROJ_INTERLEAVE:
                for p in range(4):
                    for j in range(8):
                        kp = psB.tile([128, 512], f32, tag="psB", name="kp")
                        for dc in range(4):
                            nc.tensor.matmul(
                                kp, wk[:, dc, 128 * p:128 * p + 128],
                                xT[:, dc, 512 * j:512 * j + 512],
                                start=(dc == 0), stop=(dc == 3))
                        tmp = tp.tile([128, 512], f16, tag="tmp")
                        if KEVAC_DVE:
                            nc.vector.tensor_scalar_add(tmp, kp, bkp[:, p:p + 1])
                        else:
                            nc.scalar.activation(
                                out=tmp, in_=kp, func=AF.Identity,
                                bias=bkp[:, p:p + 1], scale=1.0)
                        shift_eng.dma_start(
                            out=kT_aug[0:64, 2 * p, 512 * j:512 * j + 512],
                            in_=tmp[0:64, :])
                        shift_eng.dma_start(
                            out=kT_aug[0:64, 2 * p + 1, 512 * j:512 * j + 512],
                            in_=tmp[64:128, :])
            def emit_vproj():
                for mc in range(MSP):
                    if VPROJ_AB and mc % 2 == 0:
                        vp = psA.tile([128, 1024], f32, tag="psA",
                                      name="vpa")[:, 0:512]
                    else:
                        vp = psB.tile([128, 512], f32, tag="psB", name="vp")
                    for dc in range(4):
                        nc.tensor.matmul(
                            vp, xT[:, dc, 128 * mc:128 * mc + 128],
                            wv[:, dc, :], start=(dc == 0), stop=(dc == 3))
                    vsrc = vp.rearrange("p (h d) -> p h d", h=H)
                    if EVAC_DVE:
                        nc.vector.tensor_copy(out=v_sb[:, mc, :, 0:64], in_=vsrc)
                    else:
                        nc.scalar.activation(out=v_sb[:, mc, :, 0:64],
                                             in_=vsrc, func=AF.Copy)
            if VPROJ_AB or not V_AFTER_STATS:
                emit_vproj()
            if not PROJ_INTERLEAVE:
                for p in range(0 if not STARTUP_IL else 4, 4):
                    emit_qpair(p)

            # ---- stats: row maxes -> -mu row of qT_aug
            def emit_stats(h):
                stmax = sp.tile([128, 4, 4], f32, tag="stmax")
                for lc in range(4):
                    for i in range(4):
                        st = psA.tile([128, 1024], f32, tag="psA")
                        for k in range(2):
                            m0 = 1024 * i + 512 * k
                            nc.tensor.matmul(
                                st[:, 512 * k:512 * k + 512],
                                qT_aug[0:64, h, 128 * lc:128 * lc + 128],
                                kT_aug[0:64, h, m0:m0 + 512],
                                start=True, stop=True)
                        nc.vector.reduce_max(
                            stmax[:, lc, i:i + 1], st,
                            axis=mybir.AxisListType.X)
                for lc in range(4):
                    mucol = sp.tile([128, 1], f16, tag="mucol")
                    nc.vector.reduce_max(mucol, stmax[:, lc, :],
                                         axis=mybir.AxisListType.X)
                    mpool = psO if PSO2 else psM_pool
                    murow = mpool.tile([128, 128], f32,
                                       tag="psO" if PSO2 else "psM",
                                       name="murow")
                    nc.tensor.matmul(murow[64:65, :], mucol, negI,
                                     start=True, stop=True)
                    if MUEVAC_DVE:
                        nc.vector.tensor_copy(
                            out=qT_aug[64:65, h, 128 * lc:128 * lc + 128],
                            in_=murow[64:65, :])
                    else:
                        nc.scalar.activation(
                            out=qT_aug[64:65, h, 128 * lc:128 * lc + 128],
                            in_=murow[64:65, :], func=AF.Copy)

            # ---- per-head scores^T + fastexp + PV
            def emit_exp(dst, sT, h=99):
                exp_ctr[0] += 1
                if h >= EXP_DVE_FROM_H or (
                        EXP_DVE_EVERY and exp_ctr[0] % EXP_DVE_EVERY == 0):
                    nc.vector.tensor_scalar(
                        out=dst, in0=sT, scalar1=expA, scalar2=expB,
                        op0=ALU.mult, op1=ALU.add)
                else:
                    nc.scalar.activation(out=dst, in_=sT, func=AF.Identity,
                                         bias=biasB, scale=expA)

            def emit_scores(h, fillers=None):
                oT = psO.tile([65, 512], f32, tag="psO")
                pv_queue = []
                for mcp in range(MSP // 2):
                    if fillers:
                        fillers.pop(0)()
                    et = t32p.tile([128, 1024], edt, tag="t32", name="et")
                    if EXP_PAIR:
                        sTp = psB.tile([128, 1024], f32, tag="psB", name="sTp")
                        for k in range(2):
                            mc = 2 * mcp + k
                            nc.tensor.matmul(
                                sTp[:, 512 * k:512 * k + 512],
                                kT_aug[:, h, 128 * mc:128 * mc + 128],
                                qT_aug[:, h, :], start=True, stop=True)
                        emit_exp(et, sTp, h)
                    else:
                        for k in range(2):
                            mc = 2 * mcp + k
                            if ST_ALT3 and mc % 3 == 2:
                                sT = psM_pool.tile([128, 512], f32, tag="psM",
                                                   name="sTm")
                            else:
                                sT = psB.tile([128, 512], f32, tag="psB",
                                              name="sT")
                            nc.tensor.matmul(
                                sT, kT_aug[:, h, 128 * mc:128 * mc + 128],
                                qT_aug[:, h, :], start=True, stop=True)
                            emit_exp(et[:, 512 * k:512 * k + 512], sT, h)
                    if f16mode:
                        e16 = e16p.tile([128, 1024], f16, tag="e16")
                        nc.gpsimd.tensor_copy(out=e16, in_=et.bitcast(f32))
                        esrc = e16
                    else:
                        esrc = et.bitcast(bf16 if EMODE == "bf16" else f16)
                    pv_queue.append((mcp, esrc))
                    if len(pv_queue) >= 2:
                        _emit_pv(pv_queue.pop(0), oT, h)
                while pv_queue:
                    _emit_pv(pv_queue.pop(0), oT, h)
                return oT

            def _emit_pv(item, oT, h):
                mcp, esrc = item
                for k in range(2):
                    mc = 2 * mcp + k
                    nc.tensor.matmul(oT, v_sb[:, mc, h, :],
                                     esrc[:, 512 * k:512 * k + 512],
                                     start=(mc == 0), stop=(mc == MSP - 1))

            # ---- out stage
            def emit_out(h, oT):
                oT_sb = sp.tile([65, 512], f32, tag="oTsb")
                if OTEVAC_DVE:
                    nc.vector.tensor_copy(out=oT_sb, in_=oT)
                else:
                    nc.scalar.activation(out=oT_sb, in_=oT, func=AF.Copy)
                last = h == H - 1
                for lc in range(4):
                    # for the final head, alternate with the freed psO bank so
                    # two lc-chains can be in flight
                    use_o = PSO2 or (last and lc % 2 == 1)
                    mpool = psO if use_o else psM_pool
                    mtag = "psO" if use_o else "psM"
                    onat = mpool.tile([128, 128], f32, tag=mtag,
                                      name="onat")
                    nc.tensor.transpose(
                        onat[:, 0:65], oT_sb[:, 128 * lc:128 * lc + 128],
                        idf32[0:65, 0:65])
                    rcol = sp.tile([128, 1], f32, tag="rcol")
                    nc.vector.reciprocal(rcol, onat[:, 64:65])
                    anat = sp.tile([128, 64], f16, tag="anat")
                    nc.vector.tensor_scalar_mul(anat, onat[:, 0:64], rcol)
                    aT = mpool.tile([128, 128], f16, tag=mtag, name="aT")
                    hb = 64 * (h % 2)
                    nc.tensor.transpose(aT[hb:hb + 64, 0:128], anat, idf16)
                    if ATEVAC_DVE:
                        nc.vector.tensor_copy(
                            out=attnT[hb:hb + 64, h // 2,
                                      128 * lc:128 * lc + 128],
                            in_=aT[hb:hb + 64, 0:128])
                    else:
                        nc.scalar.activation(
                            out=attnT[hb:hb + 64, h // 2,
                                      128 * lc:128 * lc + 128],
                            in_=aT[hb:hb + 64, 0:128], func=AF.Copy)

            if PROJ_INTERLEAVE:
                for p in range(4):
                    for jp in range(4):
                        emit_kpair(p, jp)
                    emit_qpair(p)
                    emit_stats(2 * p)
                    emit_stats(2 * p + 1)
                emit_vproj()
            else:
                emit_stats(0)
                emit_stats(1)
                if V_AFTER_STATS and not VPROJ_AB:
                    emit_vproj()
            prev = None
            y_tiles = []

            def y_slot(lc):
                return y_tiles[lc // 2][:, 512 * (lc % 2):512 * (lc % 2) + 512]

            def emit_y_partial(p, start, stop):
                for lc in range(4):
                    nc.tensor.matmul(
                        y_slot(lc),
                        attnT[:, p, 128 * lc:128 * lc + 128],
                        wo[:, p, :], start=start, stop=stop)

            for h in range(H):
                if Y_EARLY and h == H - 1:
                    # stats all done; psA is free. Hold both psA slots with
                    # partial y accumulations as PE filler for the ACT-bound
                    # last head.
                    for s in range(2):
                        y_tiles.append(psA.tile([128, 1024], f32, tag="psA",
                                                name="yearly"))
                    fillers = []
                    for p in range(3):
                        for lc in range(4):
                            def mk(p=p, lc=lc):
                                nc.tensor.matmul(
                                    y_slot(lc),
                                    attnT[:, p, 128 * lc:128 * lc + 128],
                                    wo[:, p, :], start=(p == 0), stop=False)
                            fillers.append(mk)
                    oT = emit_scores(h, fillers)
                else:
                    oT = emit_scores(h)
                if not PROJ_INTERLEAVE and h + 2 < H:
                    emit_stats(h + 2)
                if prev is not None:
                    emit_out(prev[0], prev[1])
                prev = (h, oT)
            emit_out(prev[0], prev[1])

            # ---- output projection (tail; early partials in h=7 round)
            for lc in range(4):
                if Y_EARLY:
                    yp = y_slot(lc)
                    nc.tensor.matmul(
                        yp, attnT[:, 3, 128 * lc:128 * lc + 128],
                        wo[:, 3, :], start=False, stop=True)
                else:
                    yp = psA.tile([128, 1024], f32, tag="psA")
                    for p in range(4):
                        nc.tensor.matmul(
                            yp[:, 0:512],
                            attnT[:, p, 128 * lc:128 * lc + 128],
                            wo[:, p, :], start=(p == 0), stop=(p == 3))
                y_sb = sp.tile([128, DIM], f32, tag="ysb")
                nc.vector.tensor_add(
                    y_sb, yp if Y_EARLY else yp[:, 0:512], crow_b)
                nc.sync.dma_start(out=d_y[128 * lc:128 * lc + 128, :],
                                  in_=y_sb)

    nc.compile()
    _CACHE[key] = nc
    return nc


def prep_in_maps(x, Wq, bq, Wk, bk, Wv, bv, Wout, tgt_len):
    assert int(tgt_len) == SEQ
    f16 = np.float16
    f32c = lambda a: np.asarray(a, dtype=np.float32)

    x, Wq, bq, Wk, bk = f32c(x), f32c(Wq), f32c(bq), f32c(Wk), f32c(bk)
    Wv, bv, Wout = f32c(Wv), f32c(bv), f32c(Wout)

    def chunk4(a, w):  # [512, w] -> [128, 4, w]
        return np.ascontiguousarray(
            a.reshape(4, 128, w).transpose(1, 0, 2))

    xT = np.ascontiguousarray(x.T)
    xT16 = chunk4(xT, SEQ).astype(f16)
    wq16 = chunk4(np.ascontiguousarray(Wq.T) * np.float32(SCALING),
                  DIM).astype(f16)
    wk16 = chunk4(np.ascontiguousarray(Wk.T), DIM).astype(f16)
    wv16 = chunk4(np.ascontiguousarray(Wv.T), DIM).astype(f16)
    wo16 = chunk4(np.ascontiguousarray(Wout.T), DIM).astype(f16)
    bqp = np.ascontiguousarray((bq * np.float32(SCALING)).reshape(4, 128).T)
    bkp = np.ascontiguousarray(bk.reshape(4, 128).T)
    crow = np.ascontiguousarray(Wout @ bv).astype(np.float32)
    ones16 = np.ones((H, SEQ), f16)

    in_maps = []
    for c in range(NCORES):
        xq16 = np.ascontiguousarray(xT16[:, :, LSP * c:LSP * (c + 1)])
        in_maps.append({
            "xT": xT16, "xqT": xq16, "wq": wq16, "wk": wk16, "wv": wv16,
            "wo": wo16, "bqp": bqp, "bkp": bkp, "crow": crow,
            "ones16": ones16,
        })
    return in_maps


def kernel(**inputs):
    from concourse.bass_utils import run_bass_kernel_spmd
    in_maps = prep_in_maps(**inputs)
    nc = _build()
    res = run_bass_kernel_spmd(nc, in_maps, core_ids=list(range(NCORES)))
    y = np.concatenate([r["y"] for r in res.results], axis=0)
    return y.astype(np.float32)



# revision 5
# speedup vs baseline: 1.0162x; 1.0162x over previous
"""Trainium2 Bass kernel for MultiHeadedSelfAttention (fastexp softmax).

Sharding: sequence-parallel over 8 cores. Each core computes K/V for the
full sequence and attention for its own 512-row query block; outputs are
disjoint row blocks of the final [4096, 512] result, so no collectives.

Device layout is "transposed everything": activations stored feature-major
(d on partitions) so projections and scores feed the PE contraction dim
directly. Softmax row-max is computed from an [l, m]-oriented score pass
(DVE reduce), then scores are recomputed transposed [m, l] with the row max
injected as an extra contraction row (ones x -mu), so the exponent input
arrives in PSUM already max-subtracted. The Schraudolph fastexp is applied
bit-exactly: ACT affine (scale=A, bias=B) with int32 output = the reference's
int(A*x+B); the int32 bit pattern viewed as f32 is e. GPSIMD converts e to
fp16 for the PV matmul. Row sums come free as a ones column appended to V.
"""

import numpy as np

DIM = 512
H = 8
HD = 64
SEQ = 4096
NCORES = 8
LSP = SEQ // NCORES  # 512 query rows per core

GIST_A = 12102203.17133801
GIST_B = 1064986823.0


def _q_rsqrt(x):
    y = np.asarray((x,), dtype=np.float32)
    x2 = y * 0.5
    i = y.view(np.int32)
    i = np.right_shift(i, 1)
    i = 1597463007 - i
    y = i.view(np.float32)
    y = y * (1.5 - x2 * y * y)
    return float(y[0])


SCALING = _q_rsqrt(HD)

_CACHE = {}


EMODE = "f16t"         # "f16": GP-converted fp16 e; "bf16": int16-trick bf16 e
STATS_SUB = 4          # row-max from every-Nth score column (1 = exact)
EXP_DVE_EVERY = 8      # every Nth exp tile on DVE instead of ACT (0 = all ACT)
EVAC_DVE = False       # v evacuations on DVE instead of ACT
KEVAC_DVE = False      # kT evacuations on DVE (tensor_scalar add)
XT_SPLIT = False       # split xT input DMA into 4 chunk DMAs
PSO2 = False           # merge psM into psO with bufs=2
EXP_PAIR = False       # exp per [128,1024] psum span (psB coarse, psA bufs=1)
ATEVAC_DVE = True      # aT evacuation on DVE
OTEVAC_DVE = False     # oT evacuation on DVE
MUEVAC_DVE = True      # mu-row evacuation on DVE
SHIFT_GPDMA = False    # kT/qT partition-shift DMAs via SWDGE (gpsimd)
DMA_REORDER = True     # wk first, xT split into chunks
VPROJ_AB = False       # v-proj alternates psA/psB slots, before stats
DMA_FINE = False       # wk per-dc + xT half-chunks, alternating DMA engines
ST_ALT3 = False        # every 3rd sT tile allocates from psM (3-deep sT)
XT_JMAJOR = False      # xT DMA in 8 j-span slices (consumption order)
KSHIFT_PAIR = True     # kT evac+shift in j-pairs (32 shift DMAs, tmp 1024)
PROJ_INTERLEAVE = False  # p-outer kT loop; q+stats emitted per pair
EXP_DVE_FROM_H = 5     # route exp to DVE for heads >= this (DVE idle tail)
Y_EARLY = False        # emit y pair-partials 0-2 as fillers in the h=7 round
V_AFTER_STATS = True   # emit v projection after stats(0)/stats(1)
STARTUP_IL = True      # dc-interleave the first 8 kT tiles across idle psum
KSHIFT_QUAD = False    # 4-span kT staging (fewer shift DMAs, tmp bufs 3)


def _build():
    key = ("nc", EMODE, EXP_DVE_EVERY, EVAC_DVE, V_AFTER_STATS, KEVAC_DVE, XT_SPLIT, PSO2, EXP_PAIR, ATEVAC_DVE, OTEVAC_DVE, MUEVAC_DVE, SHIFT_GPDMA, DMA_REORDER, VPROJ_AB, DMA_FINE, ST_ALT3, XT_JMAJOR, KSHIFT_PAIR, PROJ_INTERLEAVE, EXP_DVE_FROM_H, Y_EARLY, STARTUP_IL, KSHIFT_QUAD, STATS_SUB)
    if key in _CACHE:
        return _CACHE[key]

    import concourse.bass as bass
    import concourse.mybir as mybir
    import concourse.tile as tile
    from concourse import bacc
    from concourse.masks import make_identity

    f16 = mybir.dt.float16
    bf16 = mybir.dt.bfloat16
    f32 = mybir.dt.float32
    i32 = mybir.dt.int32
    i16 = mybir.dt.int16
    AF = mybir.ActivationFunctionType
    ALU = mybir.AluOpType

    f16mode = EMODE == "f16"
    if f16mode:
        vdt, edt, expA, expB = f16, i32, GIST_A, GIST_B
    elif EMODE == "bf16":
        vdt, edt = bf16, i16
        expA, expB = GIST_A / 65536.0, GIST_B / 65536.0
    else:  # "f16t": Schraudolph applied directly in the fp16 bit domain
        vdt, edt = f16, i16
        expA = 1024.0 / float(np.log(2.0))
        expB = 15.0 * 1024.0 + (GIST_B / 8192.0 - 130048.0)

    nc = bacc.Bacc("TRN2", target_bir_lowering=False, debug=False,
                   num_devices=NCORES)

    d_xT = nc.dram_tensor("xT", (128, 4, SEQ), f16, kind="ExternalInput")
    d_xqT = nc.dram_tensor("xqT", (128, 4, LSP), f16, kind="ExternalInput")
    d_wq = nc.dram_tensor("wq", (128, 4, DIM), f16, kind="ExternalInput")
    d_wk = nc.dram_tensor("wk", (128, 4, DIM), f16, kind="ExternalInput")
    d_wv = nc.dram_tensor("wv", (128, 4, DIM), f16, kind="ExternalInput")
    d_wo = nc.dram_tensor("wo", (128, 4, DIM), f16, kind="ExternalInput")
    d_bqp = nc.dram_tensor("bqp", (128, 4), f32, kind="ExternalInput")
    d_bkp = nc.dram_tensor("bkp", (128, 4), f32, kind="ExternalInput")
    d_crow = nc.dram_tensor("crow", (DIM,), f32, kind="ExternalInput")
    d_ones = nc.dram_tensor("ones16", (H, SEQ), f16, kind="ExternalInput")
    d_y = nc.dram_tensor("y", (LSP, DIM), f32, kind="ExternalOutput")

    MSP = SEQ // 128  # 32 m chunks
    exp_ctr = [0]

    with tile.TileContext(nc) as tc:
        with (
            tc.tile_pool(name="const", bufs=1) as cp,
            tc.tile_pool(name="big", bufs=1) as bp,
            tc.tile_pool(name="tmp", bufs=3 if KSHIFT_QUAD else 6) as tp,
            tc.tile_pool(name="small", bufs=4) as sp,
            tc.tile_pool(name="t32p", bufs=4) as t32p,
            tc.tile_pool(name="e16p", bufs=3) as e16p,
            tc.tile_pool(name="psA", bufs=1 if EXP_PAIR else 2,
                         space="PSUM") as psA,
            tc.tile_pool(name="psB", bufs=2, space="PSUM") as psB,
            tc.tile_pool(name="psO", bufs=2 if PSO2 else 1,
                         space="PSUM") as psO,
            tc.tile_pool(name="psM", bufs=1, space="PSUM") as psM_pool,
        ):
            # ---- constants / inputs in SBUF
            kT_aug = bp.tile([65, H, SEQ], f16)     # per-head k^T + ones row
            xT = cp.tile([128, 4, SEQ], f16)
            xqT = cp.tile([128, 4, LSP], f16)
            wq = cp.tile([128, 4, DIM], f16)
            wk = cp.tile([128, 4, DIM], f16)
            wv = cp.tile([128, 4, DIM], f16)
            wo = cp.tile([128, 4, DIM], f16)
            bqp = cp.tile([128, 4], f32)
            bkp = cp.tile([128, 4], f32)
            crow_b = cp.tile([128, DIM], f32)
            biasB = cp.tile([128, 1], f32)
            idf32 = cp.tile([128, 128], f32)
            idf16 = cp.tile([128, 128], f16)
            negI = cp.tile([128, 128], f16)

            if DMA_FINE:
                for dc in range(4):
                    nc.sync.dma_start(out=wk[:, dc, :], in_=d_wk[:, dc, :])
                for dc in range(4):
                    for hh in range(2):
                        eng = nc.sync if (dc + hh) % 2 == 0 else nc.gpsimd
                        sl = slice(2048 * hh, 2048 * hh + 2048)
                        eng.dma_start(out=xT[:, dc, sl], in_=d_xT[:, dc, sl])
                nc.sync.dma_start(out=wv, in_=d_wv[:, :, :])
                nc.sync.dma_start(out=wq, in_=d_wq[:, :, :])
                nc.sync.dma_start(out=xqT, in_=d_xqT[:, :, :])
                nc.sync.dma_start(out=wo, in_=d_wo[:, :, :])
            elif DMA_REORDER:
                nc.sync.dma_start(out=wk, in_=d_wk[:, :, :])
                nc.sync.dma_start(out=bkp, in_=d_bkp[:, :])
                if STARTUP_IL:
                    nc.sync.dma_start(out=wq, in_=d_wq[:, :, :])
                    nc.sync.dma_start(out=bqp, in_=d_bqp[:, :])
                    nc.sync.dma_start(out=xqT, in_=d_xqT[:, :, :])
                    nc.sync.dma_start(out=kT_aug[64:65, :, :], in_=d_ones[:, :])
                if XT_JMAJOR:
                    for j in range(8):
                        sl = slice(512 * j, 512 * j + 512)
                        nc.sync.dma_start(out=xT[:, :, sl], in_=d_xT[:, :, sl])
                else:
                    for dc in range(4):
                        nc.sync.dma_start(out=xT[:, dc, :], in_=d_xT[:, dc, :])
                nc.sync.dma_start(out=wv, in_=d_wv[:, :, :])
                if not STARTUP_IL:
                    nc.sync.dma_start(out=wq, in_=d_wq[:, :, :])
                    nc.sync.dma_start(out=xqT, in_=d_xqT[:, :, :])
                nc.sync.dma_start(out=wo, in_=d_wo[:, :, :])
            else:
                if XT_SPLIT:
                    for dc in range(4):
                        nc.sync.dma_start(out=xT[:, dc, :], in_=d_xT[:, dc, :])
                else:
                    nc.sync.dma_start(out=xT, in_=d_xT[:, :, :])
                nc.sync.dma_start(out=xqT, in_=d_xqT[:, :, :])
                nc.sync.dma_start(out=wq, in_=d_wq[:, :, :])
                nc.sync.dma_start(out=wk, in_=d_wk[:, :, :])
                nc.sync.dma_start(out=wv, in_=d_wv[:, :, :])
                nc.sync.dma_start(out=wo, in_=d_wo[:, :, :])
            if not DMA_REORDER:
                nc.sync.dma_start(out=bqp, in_=d_bqp[:, :])
                nc.sync.dma_start(out=bkp, in_=d_bkp[:, :])
            elif not STARTUP_IL:
                nc.sync.dma_start(out=bqp, in_=d_bqp[:, :])
            crow_ap = d_crow[:]
            crow_bcast = bass.AP(tensor=crow_ap.tensor, offset=crow_ap.offset,
                                 ap=[[0, 128]] + list(crow_ap.ap))
            nc.sync.dma_start(out=crow_b, in_=crow_bcast)

            nc.vector.memset(biasB, expB)
            make_identity(nc, idf32)
            make_identity(nc, idf16)
            nc.scalar.mul(negI, idf16, -1.0)

            # ---- persistent activations (kT_aug declared above the DMAs)
            qT_aug = bp.tile([65, H, LSP], f16)     # per-head q^T + (-mu) row
            v_sb = bp.tile([128, MSP, H, 65], vdt)  # v + ones col, m-major
            attnT = bp.tile([128, 4, LSP], f16)
            if not (DMA_REORDER and STARTUP_IL):
                nc.sync.dma_start(out=kT_aug[64:65, :, :], in_=d_ones[:, :])
            nc.vector.memset(v_sb[:, :, :, 64:65], 1.0)

            # ---- projections
            shift_eng = nc.gpsimd if SHIFT_GPDMA else nc.sync

            def emit_kpair(p, jp):
                # jp now indexes a 4-span group when KSHIFT_QUAD
                width = 4 if KSHIFT_QUAD else 2
                tmp = tp.tile([128, 512 * width], f16, tag="tmp", name="tmpk")
                for k in range(width):
                    j = width * jp + k
                    kp = psB.tile([128, 512], f32, tag="psB", name="kp")
                    for dc in range(4):
                        nc.tensor.matmul(
                            kp, wk[:, dc, 128 * p:128 * p + 128],
                            xT[:, dc, 512 * j:512 * j + 512],
                            start=(dc == 0), stop=(dc == 3))
                    nc.scalar.activation(
                        out=tmp[:, 512 * k:512 * k + 512], in_=kp,
                        func=AF.Identity, bias=bkp[:, p:p + 1],
                        scale=1.0)
                w512 = 512 * (4 if KSHIFT_QUAD else 2)
                sl = slice(w512 * jp, w512 * jp + w512)
                shift_eng.dma_start(out=kT_aug[0:64, 2 * p, sl],
                                    in_=tmp[0:64, :])
                shift_eng.dma_start(out=kT_aug[0:64, 2 * p + 1, sl],
                                    in_=tmp[64:128, :])

            def emit_kquad_rest(p):
                for j0, width in ((2, 4), (6, 2)):
                    tmp = tp.tile([128, 512 * width], f16, tag="tmp",
                                  name="tmpk")
                    for k in range(width):
                        j = j0 + k
                        kp = psB.tile([128, 512], f32, tag="psB", name="kp")
                        for dc in range(4):
                            nc.tensor.matmul(
                                kp, wk[:, dc, 128 * p:128 * p + 128],
                                xT[:, dc, 512 * j:512 * j + 512],
                                start=(dc == 0), stop=(dc == 3))
                        nc.scalar.activation(
                            out=tmp[:, 512 * k:512 * k + 512], in_=kp,
                            func=AF.Identity, bias=bkp[:, p:p + 1],
                            scale=1.0)
                    sl = slice(512 * j0, 512 * (j0 + width))
                    shift_eng.dma_start(out=kT_aug[0:64, 2 * p, sl],
                                        in_=tmp[0:64, :])
                    shift_eng.dma_start(out=kT_aug[0:64, 2 * p + 1, sl],
                                        in_=tmp[64:128, :])

            def emit_qpair(p):
                qp = psB.tile([128, 512], f32, tag="psB", name="qp")
                for dc in range(4):
                    nc.tensor.matmul(
                        qp, wq[:, dc, 128 * p:128 * p + 128],
                        xqT[:, dc, :], start=(dc == 0), stop=(dc == 3))
                tmp = tp.tile([128, 512], f16, tag="tmp")
                nc.scalar.activation(
                    out=tmp, in_=qp, func=AF.Identity,
                    bias=bqp[:, p:p + 1], scale=1.0)
                shift_eng.dma_start(out=qT_aug[0:64, 2 * p, :], in_=tmp[0:64, :])
                shift_eng.dma_start(out=qT_aug[0:64, 2 * p + 1, :],
                                    in_=tmp[64:128, :])
            if KSHIFT_PAIR and not PROJ_INTERLEAVE:
                if STARTUP_IL:
                    for p in range(4):
                        emit_qpair(p)
                    # jp=0 for all p: 8 tiles accumulate dc-interleaved so PE
                    # consumes each arriving xT chunk instead of waiting for
                    # the last one; borrows every pre-stats-idle psum bank.
                    f8 = []
                    psa_slots = [psA.tile([128, 1024], f32, tag="psA",
                                          name="ila"),
                                 psA.tile([128, 1024], f32, tag="psA",
                                          name="ilb")]
                    for p in range(4):
                        for k in range(2):
                            idx = 2 * p + k
                            if idx < 4:
                                t = psa_slots[idx // 2][:, 512 * (idx % 2):
                                                        512 * (idx % 2) + 512]
                            elif idx < 6:
                                t = psB.tile([128, 512], f32, tag="psB",
                                             name="ilc")
                            elif idx == 6:
                                t = psM_pool.tile([128, 512], f32, tag="psM",
                                                  name="ild")
                            else:
                                t = psO.tile([128, 512], f32, tag="psO",
                                             name="ile")
                            f8.append((t, p, k))
                    for dc in range(4):
                        for t, p, k in f8:
                            nc.tensor.matmul(
                                t, wk[:, dc, 128 * p:128 * p + 128],
                                xT[:, dc, 512 * k:512 * k + 512],
                                start=(dc == 0), stop=(dc == 3))
                    for p in range(4):
                        tmp = tp.tile([128, 1024], f16, tag="tmp", name="tmpk")
                        for k in range(2):
                            t = f8[2 * p + k][0]
                            nc.scalar.activation(
                                out=tmp[:, 512 * k:512 * k + 512], in_=t,
                                func=AF.Identity, bias=bkp[:, p:p + 1],
                                scale=1.0)
                        shift_eng.dma_start(out=kT_aug[0:64, 2 * p, 0:1024],
                                            in_=tmp[0:64, :])
                        shift_eng.dma_start(out=kT_aug[0:64, 2 * p + 1, 0:1024],
                                            in_=tmp[64:128, :])
                    if KSHIFT_QUAD:
                        # spans 2..7 remain: one quad group (2-5) + one pair-ish
                        # handled as group starting at jp such that width*jp=2
                        # simpler: two groups of width 3 not supported; use
                        # per-p: one [2..5] quad then [6..7] pair via direct
                        for p in range(4):
                            emit_kquad_rest(p)
                    else:
                        for p in range(4):
                            for jp in range(1, 4):
                                emit_kpair(p, jp)
                else:
                    for jp in range(4):   # j-pairs; j-outer for DMA arrival
                        for p in range(4):
                            emit_kpair(p, jp)
            elif not PROJ_INTERLEAVE:
                for p in range(4):
                    for j in range(8):
                        kp = psB.tile([128, 512], f32, tag="psB", name="kp")
                        for dc in range(4):
                            nc.tensor.matmul(
                                kp, wk[:, dc, 128 * p:128 * p + 128],
                                xT[:, dc, 512 * j:512 * j + 512],
                                start=(dc == 0), stop=(dc == 3))
                        tmp = tp.tile([128, 512], f16, tag="tmp")
                        if KEVAC_DVE:
                            nc.vector.tensor_scalar_add(tmp, kp, bkp[:, p:p + 1])
                        else:
                            nc.scalar.activation(
                                out=tmp, in_=kp, func=AF.Identity,
                                bias=bkp[:, p:p + 1], scale=1.0)
                        shift_eng.dma_start(
                            out=kT_aug[0:64, 2 * p, 512 * j:512 * j + 512],
                            in_=tmp[0:64, :])
                        shift_eng.dma_start(
                            out=kT_aug[0:64, 2 * p + 1, 512 * j:512 * j + 512],
                            in_=tmp[64:128, :])
            def emit_vproj():
                for mc in range(MSP):
                    if VPROJ_AB and mc % 2 == 0:
                        vp = psA.tile([128, 1024], f32, tag="psA",
                                      name="vpa")[:, 0:512]
                    else:
                        vp = psB.tile([128, 512], f32, tag="psB", name="vp")
                    for dc in range(4):
                        nc.tensor.matmul(
                            vp, xT[:, dc, 128 * mc:128 * mc + 128],
                            wv[:, dc, :], start=(dc == 0), stop=(dc == 3))
                    vsrc = vp.rearrange("p (h d) -> p h d", h=H)
                    if EVAC_DVE:
                        nc.vector.tensor_copy(out=v_sb[:, mc, :, 0:64], in_=vsrc)
                    else:
                        nc.scalar.activation(out=v_sb[:, mc, :, 0:64],
                                             in_=vsrc, func=AF.Copy)
            if VPROJ_AB or not V_AFTER_STATS:
                emit_vproj()
            if not PROJ_INTERLEAVE:
                for p in range(0 if not STARTUP_IL else 4, 4):
                    emit_qpair(p)

            # ---- stats: row maxes -> -mu row of qT_aug
            # Subsampled (stride STATS_SUB) score pass: the softmax is
            # invariant to a uniform row shift, so a slightly-low max only
            # perturbs the Schraudolph epsilon pattern (~eps' * delta).
            def emit_stats(h):
                if STATS_SUB == 1:
                    return emit_stats_full(h)
                ssub = SEQ // STATS_SUB
                half = ssub // 2
                for lc in range(4):
                    st = psA.tile([128, ssub], f32, tag="psA")
                    for k in range(2):
                        m0 = (SEQ // 2) * k
                        nc.tensor.matmul(
                            st[:, half * k:half * k + half],
                            qT_aug[0:64, h, 128 * lc:128 * lc + 128],
                            kT_aug[0:64, h, m0:m0 + SEQ // 2:STATS_SUB],
                            start=True, stop=True)
                    mucol = sp.tile([128, 1], f16, tag="mucol")
                    nc.vector.reduce_max(mucol, st, axis=mybir.AxisListType.X)
                    mpool = psO if PSO2 else psM_pool
                    murow = mpool.tile([128, 128], f32,
                                       tag="psO" if PSO2 else "psM",
                                       name="murow")
                    nc.tensor.matmul(murow[64:65, :], mucol, negI,
                                     start=True, stop=True)
                    if MUEVAC_DVE:
                        nc.vector.tensor_copy(
                            out=qT_aug[64:65, h, 128 * lc:128 * lc + 128],
                            in_=murow[64:65, :])
                    else:
                        nc.scalar.activation(
                            out=qT_aug[64:65, h, 128 * lc:128 * lc + 128],
                            in_=murow[64:65, :], func=AF.Copy)

            def emit_stats_full(h):
                stmax = sp.tile([128, 4, 4], f32, tag="stmax")
                for lc in range(4):
                    for i in range(4):
                        st = psA.tile([128, 1024], f32, tag="psA")
                        for k in range(2):
                            m0 = 1024 * i + 512 * k
                            nc.tensor.matmul(
                                st[:, 512 * k:512 * k + 512],
                                qT_aug[0:64, h, 128 * lc:128 * lc + 128],
                                kT_aug[0:64, h, m0:m0 + 512],
                                start=True, stop=True)
                        nc.vector.reduce_max(
                            stmax[:, lc, i:i + 1], st,
                            axis=mybir.AxisListType.X)
                for lc in range(4):
                    mucol = sp.tile([128, 1], f16, tag="mucol")
                    nc.vector.reduce_max(mucol, stmax[:, lc, :],
                                         axis=mybir.AxisListType.X)
                    mpool = psO if PSO2 else psM_pool
                    murow = mpool.tile([128, 128], f32,
                                       tag="psO" if PSO2 else "psM",
                                       name="murow")
                    nc.tensor.matmul(murow[64:65, :], mucol, negI,
                                     start=True, stop=True)
                    if MUEVAC_DVE:
                        nc.vector.tensor_copy(
                            out=qT_aug[64:65, h, 128 * lc:128 * lc + 128],
                            in_=murow[64:65, :])
                    else:
                        nc.scalar.activation(
                            out=qT_aug[64:65, h, 128 * lc:128 * lc + 128],
                            in_=murow[64:65, :], func=AF.Copy)

            # ---- per-head scores^T + fastexp + PV
            def emit_exp(dst, sT, h=99):
                exp_ctr[0] += 1
                if h >= EXP_DVE_FROM_H or (
                        EXP_DVE_EVERY and exp_ctr[0] % EXP_DVE_EVERY == 0):
                    nc.vector.tensor_scalar(
                        out=dst, in0=sT, scalar1=expA, scalar2=expB,
                        op0=ALU.mult, op1=ALU.add)
                else:
                    nc.scalar.activation(out=dst, in_=sT, func=AF.Identity,
                                         bias=biasB, scale=expA)

            def emit_scores(h, fillers=None):
                oT = psO.tile([65, 512], f32, tag="psO")
                pv_queue = []
                for mcp in range(MSP // 2):
                    if fillers:
                        fillers.pop(0)()
                    et = t32p.tile([128, 1024], edt, tag="t32", name="et")
                    if EXP_PAIR:
                        sTp = psB.tile([128, 1024], f32, tag="psB", name="sTp")
                        for k in range(2):
                            mc = 2 * mcp + k
                            nc.tensor.matmul(
                                sTp[:, 512 * k:512 * k + 512],
                                kT_aug[:, h, 128 * mc:128 * mc + 128],
                                qT_aug[:, h, :], start=True, stop=True)
                        emit_exp(et, sTp, h)
                    else:
                        for k in range(2):
                            mc = 2 * mcp + k
                            if ST_ALT3 and mc % 3 == 2:
                                sT = psM_pool.tile([128, 512], f32, tag="psM",
                                                   name="sTm")
                            else:
                                sT = psB.tile([128, 512], f32, tag="psB",
                                              name="sT")
                            nc.tensor.matmul(
                                sT, kT_aug[:, h, 128 * mc:128 * mc + 128],
                                qT_aug[:, h, :], start=True, stop=True)
                            emit_exp(et[:, 512 * k:512 * k + 512], sT, h)
                    if f16mode:
                        e16 = e16p.tile([128, 1024], f16, tag="e16")
                        nc.gpsimd.tensor_copy(out=e16, in_=et.bitcast(f32))
                        esrc = e16
                    else:
                        esrc = et.bitcast(bf16 if EMODE == "bf16" else f16)
                    pv_queue.append((mcp, esrc))
                    if len(pv_queue) >= 2:
                        _emit_pv(pv_queue.pop(0), oT, h)
                while pv_queue:
                    _emit_pv(pv_queue.pop(0), oT, h)
                return oT

            def _emit_pv(item, oT, h):
                mcp, esrc = item
                for k in range(2):
                    mc = 2 * mcp + k
                    nc.tensor.matmul(oT, v_sb[:, mc, h, :],
                                     esrc[:, 512 * k:512 * k + 512],
                                     start=(mc == 0), stop=(mc == MSP - 1))

            # ---- out stage
            def emit_out(h, oT):
                oT_sb = sp.tile([65, 512], f32, tag="oTsb")
                if OTEVAC_DVE:
                    nc.vector.tensor_copy(out=oT_sb, in_=oT)
                else:
                    nc.scalar.activation(out=oT_sb, in_=oT, func=AF.Copy)
                last = h == H - 1
                for lc in range(4):
                    # for the final head, alternate with the freed psO bank so
                    # two lc-chains can be in flight
                    use_o = PSO2 or (last and lc % 2 == 1)
                    mpool = psO if use_o else psM_pool
                    mtag = "psO" if use_o else "psM"
                    onat = mpool.tile([128, 128], f32, tag=mtag,
                                      name="onat")
                    nc.tensor.transpose(
                        onat[:, 0:65], oT_sb[:, 128 * lc:128 * lc + 128],
                        idf32[0:65, 0:65])
                    rcol = sp.tile([128, 1], f32, tag="rcol")
                    nc.vector.reciprocal(rcol, onat[:, 64:65])
                    anat = sp.tile([128, 64], f16, tag="anat")
                    nc.vector.tensor_scalar_mul(anat, onat[:, 0:64], rcol)
                    aT = mpool.tile([128, 128], f16, tag=mtag, name="aT")
                    hb = 64 * (h % 2)
                    nc.tensor.transpose(aT[hb:hb + 64, 0:128], anat, idf16)
                    if ATEVAC_DVE:
                        nc.vector.tensor_copy(
                            out=attnT[hb:hb + 64, h // 2,
                                      128 * lc:128 * lc + 128],
                            in_=aT[hb:hb + 64, 0:128])
                    else:
                        nc.scalar.activation(
                            out=attnT[hb:hb + 64, h // 2,
                                      128 * lc:128 * lc + 128],
                            in_=aT[hb:hb + 64, 0:128], func=AF.Copy)

            if PROJ_INTERLEAVE:
                for p in range(4):
                    for jp in range(4):
                        emit_kpair(p, jp)
                    emit_qpair(p)
                    emit_stats(2 * p)
                    emit_stats(2 * p + 1)
                emit_vproj()
            else:
                emit_stats(0)
                emit_stats(1)
                if V_AFTER_STATS and not VPROJ_AB:
                    emit_vproj()
            prev = None
            y_tiles = []

            def y_slot(lc):
                return y_tiles[lc // 2][:, 512 * (lc % 2):512 * (lc % 2) + 512]

            def emit_y_partial(p, start, stop):
                for lc in range(4):
                    nc.tensor.matmul(
                        y_slot(lc),
                        attnT[:, p, 128 * lc:128 * lc + 128],
                        wo[:, p, :], start=start, stop=stop)

            for h in range(H):
                if Y_EARLY and h == H - 1:
                    # stats all done; psA is free. Hold both psA slots with
                    # partial y accumulations as PE filler for the ACT-bound
                    # last head.
                    for s in range(2):
                        y_tiles.append(psA.tile([128, 1024], f32, tag="psA",
                                                name="yearly"))
                    fillers = []
                    for p in range(3):
                        for lc in range(4):
                            def mk(p=p, lc=lc):
                                nc.tensor.matmul(
                                    y_slot(lc),
                                    attnT[:, p, 128 * lc:128 * lc + 128],
                                    wo[:, p, :], start=(p == 0), stop=False)
                            fillers.append(mk)
                    oT = emit_scores(h, fillers)
                else:
                    oT = emit_scores(h)
                if not PROJ_INTERLEAVE and h + 2 < H:
                    emit_stats(h + 2)
                if prev is not None:
                    emit_out(prev[0], prev[1])
                prev = (h, oT)
            emit_out(prev[0], prev[1])

            # ---- output projection (tail; early partials in h=7 round)
            for lc in range(4):
                if Y_EARLY:
                    yp = y_slot(lc)
                    nc.tensor.matmul(
                        yp, attnT[:, 3, 128 * lc:128 * lc + 128],
                        wo[:, 3, :], start=False, stop=True)
                else:
                    yp = psA.tile([128, 1024], f32, tag="psA")
                    for p in range(4):
                        nc.tensor.matmul(
                            yp[:, 0:512],
                            attnT[:, p, 128 * lc:128 * lc + 128],
                            wo[:, p, :], start=(p == 0), stop=(p == 3))
                y_sb = sp.tile([128, DIM], f32, tag="ysb")
                nc.vector.tensor_add(
                    y_sb, yp if Y_EARLY else yp[:, 0:512], crow_b)
                nc.sync.dma_start(out=d_y[128 * lc:128 * lc + 128, :],
                                  in_=y_sb)

    nc.compile()
    _CACHE[key] = nc
    return nc


def prep_in_maps(x, Wq, bq, Wk, bk, Wv, bv, Wout, tgt_len):
    assert int(tgt_len) == SEQ
    f16 = np.float16
    f32c = lambda a: np.asarray(a, dtype=np.float32)

    x, Wq, bq, Wk, bk = f32c(x), f32c(Wq), f32c(bq), f32c(Wk), f32c(bk)
    Wv, bv, Wout = f32c(Wv), f32c(bv), f32c(Wout)

    def chunk4(a, w):  # [512, w] -> [128, 4, w]
        return np.ascontiguousarray(
            a.reshape(4, 128, w).transpose(1, 0, 2))

    xT = np.ascontiguousarray(x.T)
    xT16 = chunk4(xT, SEQ).astype(f16)
    wq16 = chunk4(np.ascontiguousarray(Wq.T) * np.float32(SCALING),
                  DIM).astype(f16)
    wk16 = chunk4(np.ascontiguousarray(Wk.T), DIM).astype(f16)
    wv16 = chunk4(np.ascontiguousarray(Wv.T), DIM).astype(f16)
    wo16 = chunk4(np.ascontiguousarray(Wout.T), DIM).astype(f16)
    bqp = np.ascontiguousarray((bq * np.float32(SCALING)).reshape(4, 128).T)
    bkp = np.ascontiguousarray(bk.reshape(4, 128).T)
    crow = np.ascontiguousarray(Wout @ bv).astype(np.float32)
    ones16 = np.ones((H, SEQ), f16)

    in_maps = []
    for c in range(NCORES):
        xq16 = np.ascontiguousarray(xT16[:, :, LSP * c:LSP * (c + 1)])
        in_maps.append({
            "xT": xT16, "xqT": xq16, "wq": wq16, "wk": wk16, "wv": wv16,
            "wo": wo16, "bqp": bqp, "bkp": bkp, "crow": crow,
            "ones16": ones16,
        })
    return in_maps


def kernel(**inputs):
    from concourse.bass_utils import run_bass_kernel_spmd
    in_maps = prep_in_maps(**inputs)
    nc = _build()
    res = run_bass_kernel_spmd(nc, in_maps, core_ids=list(range(NCORES)))
    y = np.concatenate([r["y"] for r in res.results], axis=0)
    return y.astype(np.float32)



# revision 7
# speedup vs baseline: 1.0520x; 1.0352x over previous
"""Trainium2 Bass kernel for MultiHeadedSelfAttention (fastexp softmax).

Sharding: sequence-parallel over 8 cores. Each core computes K/V for the
full sequence and attention for its own 512-row query block; outputs are
disjoint row blocks of the final [4096, 512] result, so no collectives.

Device layout is "transposed everything": activations stored feature-major
(d on partitions) so projections and scores feed the PE contraction dim
directly. Softmax row-max is computed from an [l, m]-oriented score pass
(DVE reduce), then scores are recomputed transposed [m, l] with the row max
injected as an extra contraction row (ones x -mu), so the exponent input
arrives in PSUM already max-subtracted. The Schraudolph fastexp is applied
bit-exactly: ACT affine (scale=A, bias=B) with int32 output = the reference's
int(A*x+B); the int32 bit pattern viewed as f32 is e. GPSIMD converts e to
fp16 for the PV matmul. Row sums come free as a ones column appended to V.
"""

import numpy as np

DIM = 512
H = 8
HD = 64
SEQ = 4096
NCORES = 8
LSP = SEQ // NCORES  # 512 query rows per core

GIST_A = 12102203.17133801
GIST_B = 1064986823.0


def _q_rsqrt(x):
    y = np.asarray((x,), dtype=np.float32)
    x2 = y * 0.5
    i = y.view(np.int32)
    i = np.right_shift(i, 1)
    i = 1597463007 - i
    y = i.view(np.float32)
    y = y * (1.5 - x2 * y * y)
    return float(y[0])


SCALING = _q_rsqrt(HD)

_CACHE = {}


EMODE = "f16t"         # "f16": GP-converted fp16 e; "bf16": int16-trick bf16 e
STATS_SUB = 4          # row-max from every-Nth score column (1 = exact)
EXP_DVE_EVERY = 3      # every Nth exp tile on DVE instead of ACT (0 = all ACT)
EVAC_DVE = False       # v evacuations on DVE instead of ACT
KEVAC_DVE = False      # kT evacuations on DVE (tensor_scalar add)
XT_SPLIT = False       # split xT input DMA into 4 chunk DMAs
PSO2 = False           # merge psM into psO with bufs=2
EXP_PAIR = False       # exp per [128,1024] psum span (psB coarse, psA bufs=1)
ATEVAC_DVE = True      # aT evacuation on DVE
OTEVAC_DVE = False     # oT evacuation on DVE
MUEVAC_DVE = True      # mu-row evacuation on DVE
SHIFT_GPDMA = False    # kT/qT partition-shift DMAs via SWDGE (gpsimd)
DMA_REORDER = True     # wk first, xT split into chunks
VPROJ_AB = False       # v-proj alternates psA/psB slots, before stats
DMA_FINE = False       # wk per-dc + xT half-chunks, alternating DMA engines
ST_ALT3 = False        # every 3rd sT tile allocates from psM (3-deep sT)
XT_JMAJOR = False      # xT DMA in 8 j-span slices (consumption order)
KSHIFT_PAIR = True     # kT evac+shift in j-pairs (32 shift DMAs, tmp 1024)
PROJ_INTERLEAVE = False  # p-outer kT loop; q+stats emitted per pair
EXP_DVE_FROM_H = 99    # route exp to DVE for heads >= this (DVE idle tail)
Y_EARLY = False        # emit y pair-partials 0-2 as fillers in the h=7 round
V_AFTER_STATS = True   # emit v projection after stats(0)/stats(1)
STARTUP_IL = True      # dc-interleave the first 8 kT tiles across idle psum
KSHIFT_QUAD = False    # 4-span kT staging (fewer shift DMAs, tmp bufs 3)


def _build():
    key = ("nc", EMODE, EXP_DVE_EVERY, EVAC_DVE, V_AFTER_STATS, KEVAC_DVE, XT_SPLIT, PSO2, EXP_PAIR, ATEVAC_DVE, OTEVAC_DVE, MUEVAC_DVE, SHIFT_GPDMA, DMA_REORDER, VPROJ_AB, DMA_FINE, ST_ALT3, XT_JMAJOR, KSHIFT_PAIR, PROJ_INTERLEAVE, EXP_DVE_FROM_H, Y_EARLY, STARTUP_IL, KSHIFT_QUAD, STATS_SUB)
    if key in _CACHE:
        return _CACHE[key]

    import concourse.bass as bass
    import concourse.mybir as mybir
    import concourse.tile as tile
    from concourse import bacc
    from concourse.masks import make_identity

    f16 = mybir.dt.float16
    bf16 = mybir.dt.bfloat16
    f32 = mybir.dt.float32
    i32 = mybir.dt.int32
    i16 = mybir.dt.int16
    AF = mybir.ActivationFunctionType
    ALU = mybir.AluOpType

    f16mode = EMODE == "f16"
    if f16mode:
        vdt, edt, expA, expB = f16, i32, GIST_A, GIST_B
    elif EMODE == "bf16":
        vdt, edt = bf16, i16
        expA, expB = GIST_A / 65536.0, GIST_B / 65536.0
    else:  # "f16t": Schraudolph applied directly in the fp16 bit domain
        vdt, edt = f16, i16
        expA = 1024.0 / float(np.log(2.0))
        expB = 15.0 * 1024.0 + (GIST_B / 8192.0 - 130048.0)

    nc = bacc.Bacc("TRN2", target_bir_lowering=False, debug=False,
                   num_devices=NCORES)

    d_xT = nc.dram_tensor("xT", (128, 4, SEQ), f16, kind="ExternalInput")
    d_xqT = nc.dram_tensor("xqT", (128, 4, LSP), f16, kind="ExternalInput")
    d_wq = nc.dram_tensor("wq", (128, 4, DIM), f16, kind="ExternalInput")
    d_wk = nc.dram_tensor("wk", (128, 4, DIM), f16, kind="ExternalInput")
    d_wv = nc.dram_tensor("wv", (128, 4, DIM), f16, kind="ExternalInput")
    d_wo = nc.dram_tensor("wo", (128, 4, DIM), f16, kind="ExternalInput")
    d_bqp = nc.dram_tensor("bqp", (128, 4), f32, kind="ExternalInput")
    d_bkp = nc.dram_tensor("bkp", (128, 4), f32, kind="ExternalInput")
    d_crow = nc.dram_tensor("crow", (DIM,), f32, kind="ExternalInput")
    d_ones = nc.dram_tensor("ones16", (H, SEQ), f16, kind="ExternalInput")
    d_y = nc.dram_tensor("y", (LSP, DIM), f32, kind="ExternalOutput")

    MSP = SEQ // 128  # 32 m chunks
    exp_ctr = [0]

    with tile.TileContext(nc) as tc:
        with (
            tc.tile_pool(name="const", bufs=1) as cp,
            tc.tile_pool(name="big", bufs=1) as bp,
            tc.tile_pool(name="tmp", bufs=3 if KSHIFT_QUAD else 6) as tp,
            tc.tile_pool(name="small", bufs=4) as sp,
            tc.tile_pool(name="t32p", bufs=4) as t32p,
            tc.tile_pool(name="e16p", bufs=3) as e16p,
            tc.tile_pool(name="psA", bufs=1 if EXP_PAIR else 2,
                         space="PSUM") as psA,
            tc.tile_pool(name="psB", bufs=2, space="PSUM") as psB,
            tc.tile_pool(name="psO", bufs=2 if PSO2 else 1,
                         space="PSUM") as psO,
            tc.tile_pool(name="psM", bufs=1, space="PSUM") as psM_pool,
        ):
            # ---- constants / inputs in SBUF
            kT_aug = bp.tile([65, H, SEQ], f16)     # per-head k^T + ones row
            xT = cp.tile([128, 4, SEQ], f16)
            xqT = cp.tile([128, 4, LSP], f16)
            wq = cp.tile([128, 4, DIM], f16)
            wk = cp.tile([128, 4, DIM], f16)
            wv = cp.tile([128, 4, DIM], f16)
            wo = cp.tile([128, 4, DIM], f16)
            bqp = cp.tile([128, 4], f32)
            bkp = cp.tile([128, 4], f32)
            crow_b = cp.tile([128, DIM], f32)
            biasB = cp.tile([128, 1], f32)
            idf32 = cp.tile([128, 128], f32)
            idf16 = cp.tile([128, 128], f16)
            negI = cp.tile([128, 128], f16)

            if DMA_FINE:
                for dc in range(4):
                    nc.sync.dma_start(out=wk[:, dc, :], in_=d_wk[:, dc, :])
                for dc in range(4):
                    for hh in range(2):
                        eng = nc.sync if (dc + hh) % 2 == 0 else nc.gpsimd
                        sl = slice(2048 * hh, 2048 * hh + 2048)
                        eng.dma_start(out=xT[:, dc, sl], in_=d_xT[:, dc, sl])
                nc.sync.dma_start(out=wv, in_=d_wv[:, :, :])
                nc.sync.dma_start(out=wq, in_=d_wq[:, :, :])
                nc.sync.dma_start(out=xqT, in_=d_xqT[:, :, :])
                nc.sync.dma_start(out=wo, in_=d_wo[:, :, :])
            elif DMA_REORDER:
                nc.sync.dma_start(out=wk, in_=d_wk[:, :, :])
                nc.sync.dma_start(out=bkp, in_=d_bkp[:, :])
                if STARTUP_IL:
                    nc.sync.dma_start(out=wq, in_=d_wq[:, :, :])
                    nc.sync.dma_start(out=bqp, in_=d_bqp[:, :])
                    nc.sync.dma_start(out=xqT, in_=d_xqT[:, :, :])
                    nc.sync.dma_start(out=kT_aug[64:65, :, :], in_=d_ones[:, :])
                if XT_JMAJOR:
                    for j in range(8):
                        sl = slice(512 * j, 512 * j + 512)
                        nc.sync.dma_start(out=xT[:, :, sl], in_=d_xT[:, :, sl])
                else:
                    for dc in range(4):
                        nc.sync.dma_start(out=xT[:, dc, :], in_=d_xT[:, dc, :])
                nc.sync.dma_start(out=wv, in_=d_wv[:, :, :])
                if not STARTUP_IL:
                    nc.sync.dma_start(out=wq, in_=d_wq[:, :, :])
                    nc.sync.dma_start(out=xqT, in_=d_xqT[:, :, :])
                nc.sync.dma_start(out=wo, in_=d_wo[:, :, :])
            else:
                if XT_SPLIT:
                    for dc in range(4):
                        nc.sync.dma_start(out=xT[:, dc, :], in_=d_xT[:, dc, :])
                else:
                    nc.sync.dma_start(out=xT, in_=d_xT[:, :, :])
                nc.sync.dma_start(out=xqT, in_=d_xqT[:, :, :])
                nc.sync.dma_start(out=wq, in_=d_wq[:, :, :])
                nc.sync.dma_start(out=wk, in_=d_wk[:, :, :])
                nc.sync.dma_start(out=wv, in_=d_wv[:, :, :])
                nc.sync.dma_start(out=wo, in_=d_wo[:, :, :])
            if not DMA_REORDER:
                nc.sync.dma_start(out=bqp, in_=d_bqp[:, :])
                nc.sync.dma_start(out=bkp, in_=d_bkp[:, :])
            elif not STARTUP_IL:
                nc.sync.dma_start(out=bqp, in_=d_bqp[:, :])
            crow_ap = d_crow[:]
            crow_bcast = bass.AP(tensor=crow_ap.tensor, offset=crow_ap.offset,
                                 ap=[[0, 128]] + list(crow_ap.ap))
            nc.sync.dma_start(out=crow_b, in_=crow_bcast)

            nc.vector.memset(biasB, expB)
            make_identity(nc, idf32)
            make_identity(nc, idf16)
            nc.scalar.mul(negI, idf16, -1.0)

            # ---- persistent activations (kT_aug declared above the DMAs)
            qT_aug = bp.tile([65, H, LSP], f16)     # per-head q^T + (-mu) row
            v_sb = bp.tile([128, MSP, H, 65], vdt)  # v + ones col, m-major
            attnT = bp.tile([128, 4, LSP], f16)
            if not (DMA_REORDER and STARTUP_IL):
                nc.sync.dma_start(out=kT_aug[64:65, :, :], in_=d_ones[:, :])
            nc.vector.memset(v_sb[:, :, :, 64:65], 1.0)

            # ---- projections
            shift_eng = nc.gpsimd if SHIFT_GPDMA else nc.sync

            def emit_kpair(p, jp):
                # jp now indexes a 4-span group when KSHIFT_QUAD
                width = 4 if KSHIFT_QUAD else 2
                tmp = tp.tile([128, 512 * width], f16, tag="tmp", name="tmpk")
                for k in range(width):
                    j = width * jp + k
                    kp = psB.tile([128, 512], f32, tag="psB", name="kp")
                    for dc in range(4):
                        nc.tensor.matmul(
                            kp, wk[:, dc, 128 * p:128 * p + 128],
                            xT[:, dc, 512 * j:512 * j + 512],
                            start=(dc == 0), stop=(dc == 3))
                    nc.scalar.activation(
                        out=tmp[:, 512 * k:512 * k + 512], in_=kp,
                        func=AF.Identity, bias=bkp[:, p:p + 1],
                        scale=1.0)
                w512 = 512 * (4 if KSHIFT_QUAD else 2)
                sl = slice(w512 * jp, w512 * jp + w512)
                shift_eng.dma_start(out=kT_aug[0:64, 2 * p, sl],
                                    in_=tmp[0:64, :])
                shift_eng.dma_start(out=kT_aug[0:64, 2 * p + 1, sl],
                                    in_=tmp[64:128, :])

            def emit_kquad_rest(p):
                for j0, width in ((2, 4), (6, 2)):
                    tmp = tp.tile([128, 512 * width], f16, tag="tmp",
                                  name="tmpk")
                    for k in range(width):
                        j = j0 + k
                        kp = psB.tile([128, 512], f32, tag="psB", name="kp")
                        for dc in range(4):
                            nc.tensor.matmul(
                                kp, wk[:, dc, 128 * p:128 * p + 128],
                                xT[:, dc, 512 * j:512 * j + 512],
                                start=(dc == 0), stop=(dc == 3))
                        nc.scalar.activation(
                            out=tmp[:, 512 * k:512 * k + 512], in_=kp,
                            func=AF.Identity, bias=bkp[:, p:p + 1],
                            scale=1.0)
                    sl = slice(512 * j0, 512 * (j0 + width))
                    shift_eng.dma_start(out=kT_aug[0:64, 2 * p, sl],
                                        in_=tmp[0:64, :])
                    shift_eng.dma_start(out=kT_aug[0:64, 2 * p + 1, sl],
                                        in_=tmp[64:128, :])

            def emit_qpair(p):
                qp = psB.tile([128, 512], f32, tag="psB", name="qp")
                for dc in range(4):
                    nc.tensor.matmul(
                        qp, wq[:, dc, 128 * p:128 * p + 128],
                        xqT[:, dc, :], start=(dc == 0), stop=(dc == 3))
                tmp = tp.tile([128, 512], f16, tag="tmp")
                nc.scalar.activation(
                    out=tmp, in_=qp, func=AF.Identity,
                    bias=bqp[:, p:p + 1], scale=1.0)
                shift_eng.dma_start(out=qT_aug[0:64, 2 * p, :], in_=tmp[0:64, :])
                shift_eng.dma_start(out=qT_aug[0:64, 2 * p + 1, :],
                                    in_=tmp[64:128, :])
            if KSHIFT_PAIR and not PROJ_INTERLEAVE:
                if STARTUP_IL:
                    for p in range(4):
                        emit_qpair(p)
                    # jp=0 for all p: 8 tiles accumulate dc-interleaved so PE
                    # consumes each arriving xT chunk instead of waiting for
                    # the last one; borrows every pre-stats-idle psum bank.
                    f8 = []
                    psa_slots = [psA.tile([128, 1024], f32, tag="psA",
                                          name="ila"),
                                 psA.tile([128, 1024], f32, tag="psA",
                                          name="ilb")]
                    for p in range(4):
                        for k in range(2):
                            idx = 2 * p + k
                            if idx < 4:
                                t = psa_slots[idx // 2][:, 512 * (idx % 2):
                                                        512 * (idx % 2) + 512]
                            elif idx < 6:
                                t = psB.tile([128, 512], f32, tag="psB",
                                             name="ilc")
                            elif idx == 6:
                                t = psM_pool.tile([128, 512], f32, tag="psM",
                                                  name="ild")
                            else:
                                t = psO.tile([128, 512], f32, tag="psO",
                                             name="ile")
                            f8.append((t, p, k))
                    for dc in range(4):
                        for t, p, k in f8:
                            nc.tensor.matmul(
                                t, wk[:, dc, 128 * p:128 * p + 128],
                                xT[:, dc, 512 * k:512 * k + 512],
                                start=(dc == 0), stop=(dc == 3))
                    for p in range(4):
                        tmp = tp.tile([128, 1024], f16, tag="tmp", name="tmpk")
                        for k in range(2):
                            t = f8[2 * p + k][0]
                            nc.scalar.activation(
                                out=tmp[:, 512 * k:512 * k + 512], in_=t,
                                func=AF.Identity, bias=bkp[:, p:p + 1],
                                scale=1.0)
                        shift_eng.dma_start(out=kT_aug[0:64, 2 * p, 0:1024],
                                            in_=tmp[0:64, :])
                        shift_eng.dma_start(out=kT_aug[0:64, 2 * p + 1, 0:1024],
                                            in_=tmp[64:128, :])
                    if KSHIFT_QUAD:
                        # spans 2..7 remain: one quad group (2-5) + one pair-ish
                        # handled as group starting at jp such that width*jp=2
                        # simpler: two groups of width 3 not supported; use
                        # per-p: one [2..5] quad then [6..7] pair via direct
                        for p in range(4):
                            emit_kquad_rest(p)
                    else:
                        for p in range(4):
                            for jp in range(1, 4):
                                emit_kpair(p, jp)
                else:
                    for jp in range(4):   # j-pairs; j-outer for DMA arrival
                        for p in range(4):
                            emit_kpair(p, jp)
            elif not PROJ_INTERLEAVE:
                for p in range(4):
                    for j in range(8):
                        kp = psB.tile([128, 512], f32, tag="psB", name="kp")
                        for dc in range(4):
                            nc.tensor.matmul(
                                kp, wk[:, dc, 128 * p:128 * p + 128],
                                xT[:, dc, 512 * j:512 * j + 512],
                                start=(dc == 0), stop=(dc == 3))
                        tmp = tp.tile([128, 512], f16, tag="tmp")
                        if KEVAC_DVE:
                            nc.vector.tensor_scalar_add(tmp, kp, bkp[:, p:p + 1])
                        else:
                            nc.scalar.activation(
                                out=tmp, in_=kp, func=AF.Identity,
                                bias=bkp[:, p:p + 1], scale=1.0)
                        shift_eng.dma_start(
                            out=kT_aug[0:64, 2 * p, 512 * j:512 * j + 512],
                            in_=tmp[0:64, :])
                        shift_eng.dma_start(
                            out=kT_aug[0:64, 2 * p + 1, 512 * j:512 * j + 512],
                            in_=tmp[64:128, :])
            def emit_vproj():
                for mc in range(MSP):
                    if VPROJ_AB and mc % 2 == 0:
                        vp = psA.tile([128, 1024], f32, tag="psA",
                                      name="vpa")[:, 0:512]
                    else:
                        vp = psB.tile([128, 512], f32, tag="psB", name="vp")
                    for dc in range(4):
                        nc.tensor.matmul(
                            vp, xT[:, dc, 128 * mc:128 * mc + 128],
                            wv[:, dc, :], start=(dc == 0), stop=(dc == 3))
                    vsrc = vp.rearrange("p (h d) -> p h d", h=H)
                    if EVAC_DVE:
                        nc.vector.tensor_copy(out=v_sb[:, mc, :, 0:64], in_=vsrc)
                    else:
                        nc.scalar.activation(out=v_sb[:, mc, :, 0:64],
                                             in_=vsrc, func=AF.Copy)
            if VPROJ_AB or not V_AFTER_STATS:
                emit_vproj()
            if not PROJ_INTERLEAVE:
                for p in range(0 if not STARTUP_IL else 4, 4):
                    emit_qpair(p)

            # ---- stats: row maxes -> -mu row of qT_aug
            # Subsampled (stride STATS_SUB) score pass: the softmax is
            # invariant to a uniform row shift, so a slightly-low max only
            # perturbs the Schraudolph epsilon pattern (~eps' * delta).
            def emit_stats(h):
                if STATS_SUB == 1:
                    return emit_stats_full(h)
                ssub = SEQ // STATS_SUB
                half = ssub // 2
                for lc in range(4):
                    st = psA.tile([128, ssub], f32, tag="psA")
                    for k in range(2):
                        m0 = (SEQ // 2) * k
                        nc.tensor.matmul(
                            st[:, half * k:half * k + half],
                            qT_aug[0:64, h, 128 * lc:128 * lc + 128],
                            kT_aug[0:64, h, m0:m0 + SEQ // 2:STATS_SUB],
                            start=True, stop=True)
                    mucol = sp.tile([128, 1], f16, tag="mucol")
                    nc.vector.reduce_max(mucol, st, axis=mybir.AxisListType.X)
                    mpool = psO if PSO2 else psM_pool
                    murow = mpool.tile([128, 128], f32,
                                       tag="psO" if PSO2 else "psM",
                                       name="murow")
                    nc.tensor.matmul(murow[64:65, :], mucol, negI,
                                     start=True, stop=True)
                    if MUEVAC_DVE:
                        nc.vector.tensor_copy(
                            out=qT_aug[64:65, h, 128 * lc:128 * lc + 128],
                            in_=murow[64:65, :])
                    else:
                        nc.scalar.activation(
                            out=qT_aug[64:65, h, 128 * lc:128 * lc + 128],
                            in_=murow[64:65, :], func=AF.Copy)

            def emit_stats_full(h):
                stmax = sp.tile([128, 4, 4], f32, tag="stmax")
                for lc in range(4):
                    for i in range(4):
                        st = psA.tile([128, 1024], f32, tag="psA")
                        for k in range(2):
                            m0 = 1024 * i + 512 * k
                            nc.tensor.matmul(
                                st[:, 512 * k:512 * k + 512],
                                qT_aug[0:64, h, 128 * lc:128 * lc + 128],
                                kT_aug[0:64, h, m0:m0 + 512],
                                start=True, stop=True)
                        nc.vector.reduce_max(
                            stmax[:, lc, i:i + 1], st,
                            axis=mybir.AxisListType.X)
                for lc in range(4):
                    mucol = sp.tile([128, 1], f16, tag="mucol")
                    nc.vector.reduce_max(mucol, stmax[:, lc, :],
                                         axis=mybir.AxisListType.X)
                    mpool = psO if PSO2 else psM_pool
                    murow = mpool.tile([128, 128], f32,
                                       tag="psO" if PSO2 else "psM",
                                       name="murow")
                    nc.tensor.matmul(murow[64:65, :], mucol, negI,
                                     start=True, stop=True)
                    if MUEVAC_DVE:
                        nc.vector.tensor_copy(
                            out=qT_aug[64:65, h, 128 * lc:128 * lc + 128],
                            in_=murow[64:65, :])
                    else:
                        nc.scalar.activation(
                            out=qT_aug[64:65, h, 128 * lc:128 * lc + 128],
                            in_=murow[64:65, :], func=AF.Copy)

            # ---- per-head scores^T + fastexp + PV
            def emit_exp(dst, sT, h=99):
                exp_ctr[0] += 1
                if h >= EXP_DVE_FROM_H or (
                        EXP_DVE_EVERY and exp_ctr[0] % EXP_DVE_EVERY == 0):
                    nc.vector.tensor_scalar(
                        out=dst, in0=sT, scalar1=expA, scalar2=expB,
                        op0=ALU.mult, op1=ALU.add)
                else:
                    nc.scalar.activation(out=dst, in_=sT, func=AF.Identity,
                                         bias=biasB, scale=expA)

            def emit_scores(h, fillers=None):
                oT = psO.tile([65, 512], f32, tag="psO")
                pv_queue = []
                for mcp in range(MSP // 2):
                    if fillers:
                        fillers.pop(0)()
                    et = t32p.tile([128, 1024], edt, tag="t32", name="et")
                    if EXP_PAIR:
                        sTp = psB.tile([128, 1024], f32, tag="psB", name="sTp")
                        for k in range(2):
                            mc = 2 * mcp + k
                            nc.tensor.matmul(
                                sTp[:, 512 * k:512 * k + 512],
                                kT_aug[:, h, 128 * mc:128 * mc + 128],
                                qT_aug[:, h, :], start=True, stop=True)
                        emit_exp(et, sTp, h)
                    else:
                        for k in range(2):
                            mc = 2 * mcp + k
                            if ST_ALT3 and mc % 3 == 2:
                                sT = psM_pool.tile([128, 512], f32, tag="psM",
                                                   name="sTm")
                            else:
                                sT = psB.tile([128, 512], f32, tag="psB",
                                              name="sT")
                            nc.tensor.matmul(
                                sT, kT_aug[:, h, 128 * mc:128 * mc + 128],
                                qT_aug[:, h, :], start=True, stop=True)
                            emit_exp(et[:, 512 * k:512 * k + 512], sT, h)
                    if f16mode:
                        e16 = e16p.tile([128, 1024], f16, tag="e16")
                        nc.gpsimd.tensor_copy(out=e16, in_=et.bitcast(f32))
                        esrc = e16
                    else:
                        esrc = et.bitcast(bf16 if EMODE == "bf16" else f16)
                    pv_queue.append((mcp, esrc))
                    if len(pv_queue) >= 2:
                        _emit_pv(pv_queue.pop(0), oT, h)
                while pv_queue:
                    _emit_pv(pv_queue.pop(0), oT, h)
                return oT

            def _emit_pv(item, oT, h):
                mcp, esrc = item
                for k in range(2):
                    mc = 2 * mcp + k
                    nc.tensor.matmul(oT, v_sb[:, mc, h, :],
                                     esrc[:, 512 * k:512 * k + 512],
                                     start=(mc == 0), stop=(mc == MSP - 1))

            # ---- out stage
            def emit_out(h, oT):
                oT_sb = sp.tile([65, 512], f32, tag="oTsb")
                if OTEVAC_DVE:
                    nc.vector.tensor_copy(out=oT_sb, in_=oT)
                else:
                    nc.scalar.activation(out=oT_sb, in_=oT, func=AF.Copy)
                last = h == H - 1
                for lc in range(4):
                    # for the final head, alternate with the freed psO bank so
                    # two lc-chains can be in flight
                    use_o = PSO2 or (last and lc % 2 == 1)
                    mpool = psO if use_o else psM_pool
                    mtag = "psO" if use_o else "psM"
                    onat = mpool.tile([128, 128], f32, tag=mtag,
                                      name="onat")
                    nc.tensor.transpose(
                        onat[:, 0:65], oT_sb[:, 128 * lc:128 * lc + 128],
                        idf32[0:65, 0:65])
                    rcol = sp.tile([128, 1], f32, tag="rcol")
                    nc.vector.reciprocal(rcol, onat[:, 64:65])
                    anat = sp.tile([128, 64], f16, tag="anat")
                    nc.vector.tensor_scalar_mul(anat, onat[:, 0:64], rcol)
                    aT = mpool.tile([128, 128], f16, tag=mtag, name="aT")
                    hb = 64 * (h % 2)
                    nc.tensor.transpose(aT[hb:hb + 64, 0:128], anat, idf16)
                    if ATEVAC_DVE:
                        nc.vector.tensor_copy(
                            out=attnT[hb:hb + 64, h // 2,
                                      128 * lc:128 * lc + 128],
                            in_=aT[hb:hb + 64, 0:128])
                    else:
                        nc.scalar.activation(
                            out=attnT[hb:hb + 64, h // 2,
                                      128 * lc:128 * lc + 128],
                            in_=aT[hb:hb + 64, 0:128], func=AF.Copy)

            if PROJ_INTERLEAVE:
                for p in range(4):
                    for jp in range(4):
                        emit_kpair(p, jp)
                    emit_qpair(p)
                    emit_stats(2 * p)
                    emit_stats(2 * p + 1)
                emit_vproj()
            else:
                emit_stats(0)
                emit_stats(1)
                if V_AFTER_STATS and not VPROJ_AB:
                    emit_vproj()
            prev = None
            y_tiles = []

            def y_slot(lc):
                return y_tiles[lc // 2][:, 512 * (lc % 2):512 * (lc % 2) + 512]

            def emit_y_partial(p, start, stop):
                for lc in range(4):
                    nc.tensor.matmul(
                        y_slot(lc),
                        attnT[:, p, 128 * lc:128 * lc + 128],
                        wo[:, p, :], start=start, stop=stop)

            for h in range(H):
                if Y_EARLY and h == H - 1:
                    # stats all done; psA is free. Hold both psA slots with
                    # partial y accumulations as PE filler for the ACT-bound
                    # last head.
                    for s in range(2):
                        y_tiles.append(psA.tile([128, 1024], f32, tag="psA",
                                                name="yearly"))
                    fillers = []
                    for p in range(3):
                        for lc in range(4):
                            def mk(p=p, lc=lc):
                                nc.tensor.matmul(
                                    y_slot(lc),
                                    attnT[:, p, 128 * lc:128 * lc + 128],
                                    wo[:, p, :], start=(p == 0), stop=False)
                            fillers.append(mk)
                    oT = emit_scores(h, fillers)
                else:
                    oT = emit_scores(h)
                if not PROJ_INTERLEAVE and h + 2 < H:
                    emit_stats(h + 2)
                if prev is not None:
                    emit_out(prev[0], prev[1])
                prev = (h, oT)
            emit_out(prev[0], prev[1])

            # ---- output projection (tail; early partials in h=7 round)
            for lc in range(4):
                if Y_EARLY:
                    yp = y_slot(lc)
                    nc.tensor.matmul(
                        yp, attnT[:, 3, 128 * lc:128 * lc + 128],
                        wo[:, 3, :], start=False, stop=True)
                else:
                    yp = psA.tile([128, 1024], f32, tag="psA")
                    for p in range(4):
                        nc.tensor.matmul(
                            yp[:, 0:512],
                            attnT[:, p, 128 * lc:128 * lc + 128],
                            wo[:, p, :], start=(p == 0), stop=(p == 3))
                y_sb = sp.tile([128, DIM], f32, tag="ysb")
                nc.vector.tensor_add(
                    y_sb, yp if Y_EARLY else yp[:, 0:512], crow_b)
                nc.sync.dma_start(out=d_y[128 * lc:128 * lc + 128, :],
                                  in_=y_sb)

    nc.compile()
    _CACHE[key] = nc
    return nc


def prep_in_maps(x, Wq, bq, Wk, bk, Wv, bv, Wout, tgt_len):
    assert int(tgt_len) == SEQ
    f16 = np.float16
    f32c = lambda a: np.asarray(a, dtype=np.float32)

    x, Wq, bq, Wk, bk = f32c(x), f32c(Wq), f32c(bq), f32c(Wk), f32c(bk)
    Wv, bv, Wout = f32c(Wv), f32c(bv), f32c(Wout)

    def chunk4(a, w):  # [512, w] -> [128, 4, w]
        return np.ascontiguousarray(
            a.reshape(4, 128, w).transpose(1, 0, 2))

    xT = np.ascontiguousarray(x.T)
    xT16 = chunk4(xT, SEQ).astype(f16)
    wq16 = chunk4(np.ascontiguousarray(Wq.T) * np.float32(SCALING),
                  DIM).astype(f16)
    wk16 = chunk4(np.ascontiguousarray(Wk.T), DIM).astype(f16)
    wv16 = chunk4(np.ascontiguousarray(Wv.T), DIM).astype(f16)
    wo16 = chunk4(np.ascontiguousarray(Wout.T), DIM).astype(f16)
    bqp = np.ascontiguousarray((bq * np.float32(SCALING)).reshape(4, 128).T)
    bkp = np.ascontiguousarray(bk.reshape(4, 128).T)
    crow = np.ascontiguousarray(Wout @ bv).astype(np.float32)
    ones16 = np.ones((H, SEQ), f16)

    in_maps = []
    for c in range(NCORES):
        xq16 = np.ascontiguousarray(xT16[:, :, LSP * c:LSP * (c + 1)])
        in_maps.append({
            "xT": xT16, "xqT": xq16, "wq": wq16, "wk": wk16, "wv": wv16,
            "wo": wo16, "bqp": bqp, "bkp": bkp, "crow": crow,
            "ones16": ones16,
        })
    return in_maps


def kernel(**inputs):
    from concourse.bass_utils import run_bass_kernel_spmd
    in_maps = prep_in_maps(**inputs)
    nc = _build()
    res = run_bass_kernel_spmd(nc, in_maps, core_ids=list(range(NCORES)))
    y = np.concatenate([r["y"] for r in res.results], axis=0)
    return y.astype(np.float32)

